# revision 1
# baseline (speedup 1.0000x reference)
"""Normalized-adjacency kernel (EstimateAdj.normalize, symmetric=False) for TRN2.

out = mx * r_inv[:, None] * r_inv[None, :]   where mx = adj + I,
r_inv = rowsum(mx) ** -0.5.

Strategy (8 NeuronCores, row-sharded, raw Bass, fp16 data movement):
  - host: mx' = (adj + I) * 2^13 cast to fp16 (the scale keeps every nonzero
    element in fp16 normal range; the net 2^26 output scale is divided back
    out on the host, so no subnormal flush can hurt relative accuracy)
  - device, per core (shard = 1024 rows x 8192 cols fp16 = 16 MiB, fully
    resident in SBUF):
      pass 1: 8 tile loads [128 x 8192] on the gpsimd ring (~335 GB/s;
              dual-ring loads measured slower at ~285 GB/s); each tile's
              rowsum is split ACT (cols 0:4480, Copy+f32 accum) / DVE
              (cols 4480:, tensor_reduce) so reduces keep pace with loads
              and the post-load tail is ~4 us.
      r_inv' = 1/sqrt(rowsum * 2^-26): DVE add halves -> ACT sqrt (fp16) ->
      PE transpose -> DVE reciprocal -> DRAM; AllGather (fp16, 2 KiB/core).
      ACT->consumer hops use the drain-publish idiom (self-wait on the
      producer's own sem, then a dummy op increments the published sem):
      a plain then_inc can fire before the engine's writebacks land, which
      produced partially-stale reads (fp16 inf) in earlier variants.
      While the AllGather is in flight, DVE pre-applies the ROW scale
      (tensor_scalar_mul, 4x mode, local r_inv') to all 16 half-tiles in
      place, so pass 2 is a plain tensor_tensor at 2x packed mode.
      colscale: partition-broadcast the gathered row to [128 x 8192].
      pass 2: DVE tensor_tensor (tile *= colscale) per half, 16 stores of
              1 MiB on the sync ring.
  - host: upcast, divide by 2^26.  Measured ~154 us (vs 326 us f32
    baseline), rel err ~2.1e-3 against the f32 reference (gate 2e-2).
    An fp8-preview variant (rowsums from an 8 MiB e4m3 copy so the
    AllGather triggers ~7 us earlier) measured equal-or-slower: HBM
    contention from its extra loads inflates the collective's trigger
    latency by the same ~7 us it saves.

(remote_dma peer-write exchange would cut the collective cost to ~5 us but
InstRemoteDMA*/hostgen variants fail neuronxcc walrus codegen on this
toolchain: "ISA wrong length" in CoreV2GenImpl visitInstISA.)
"""

from contextlib import ExitStack

import numpy as np

import concourse.bass as bass
import concourse.mybir as mybir
from concourse.bass_utils import run_bass_kernel_spmd

N = 8192
NCORES = 8
SHARD = N // NCORES  # 1024
P = 128
T = SHARD // P  # 8 tiles per core
H = 2  # column halves per tile (store/TT granularity 4096)
CA = 4480  # ACT rowsum columns (rest go to DVE)

F16 = mybir.dt.float16
F32 = mybir.dt.float32

SCALE_IN = 8192.0  # 2^13
SCALE_OUT = float(2**26)


def build_kernel(n=N, ncores=NCORES, debug=False):
    shard = n // ncores
    tt = shard // P  # 8
    w = n // H  # 4096

    nc = bass.Bass(num_devices=ncores)
    mx = nc.dram_tensor("mx", [shard, n], F16, kind="ExternalInput")
    eye = nc.dram_tensor("eye", [P, P], F16, kind="ExternalInput")
    out = nc.dram_tensor("out", [shard, n], F16, kind="ExternalOutput")
    cc_in = nc.dram_tensor("cc_in", [shard], F16)
    if debug:
        o_psa = nc.dram_tensor("o_psa", [P, 8], F32, kind="ExternalOutput")
        o_psbs = nc.dram_tensor("o_psbs", [P, 8], F32, kind="ExternalOutput")
        o_rsqh = nc.dram_tensor("o_rsqh", [P, 8], F16, kind="ExternalOutput")
        o_ccin = nc.dram_tensor("o_ccin", [shard], F16, kind="ExternalOutput")
        o_ccout = nc.dram_tensor("o_ccout", [n], F16, kind="ExternalOutput")
        o_cs = nc.dram_tensor("o_cs", [P, n], F16, kind="ExternalOutput")
    cc_out = nc.dram_tensor("cc_out", [n], F16, addr_space="Shared")

    mx_l = mx.rearrange("(t p) m -> t p m", p=P)
    out_v = out.rearrange("(t p) (h w) -> t p h w", p=P, h=H)

    with ExitStack() as ctx:
        tiles = [
            ctx.enter_context(nc.sbuf_tensor(f"tile{t}", [P, n], F16))
            for t in range(tt)
        ]
        colscale = ctx.enter_context(nc.sbuf_tensor("colscale", [P, n], F16))
        eye_sb = ctx.enter_context(nc.sbuf_tensor("eye_sb", [P, P], F16))
        psa = ctx.enter_context(nc.sbuf_tensor("psa", [P, tt], F32))
        psb = ctx.enter_context(nc.sbuf_tensor("psb", [P, tt], F32))
        psbs = ctx.enter_context(nc.sbuf_tensor("psbs", [P, tt], F32))
        ps = ctx.enter_context(nc.sbuf_tensor("ps", [P, tt], F32))
        dr1 = ctx.enter_context(nc.sbuf_tensor("dr1", [P, 1], F32))
        dr2 = ctx.enter_context(nc.sbuf_tensor("dr2", [P, 1], F16))
        rsqh = ctx.enter_context(nc.sbuf_tensor("rsqh", [P, tt], F16))
        rx8 = ctx.enter_context(nc.sbuf_tensor("rx8", [P, tt], F32))
        ptc = ctx.enter_context(nc.sbuf_tensor("ptc", [tt, P], F16))
        pt = ctx.enter_context(nc.psum_tensor("pt", [tt, P], F16))

        s_in = [ctx.enter_context(nc.semaphore(f"s_in{t}")) for t in range(tt)]
        s_eye = ctx.enter_context(nc.semaphore("s_eye"))
        s_redA = ctx.enter_context(nc.semaphore("s_redA"))
        s_psb = ctx.enter_context(nc.semaphore("s_psb"))
        s_redAd = ctx.enter_context(nc.semaphore("s_redAd"))
        s_ps = ctx.enter_context(nc.semaphore("s_ps"))
        s_sqd = ctx.enter_context(nc.semaphore("s_sqd"))
        s_sq = ctx.enter_context(nc.semaphore("s_sq"))
        s_tpl = ctx.enter_context(nc.semaphore("s_tpl"))
        s_ptc = ctx.enter_context(nc.semaphore("s_ptc"))
        s_ccin = ctx.enter_context(nc.semaphore("s_ccin"))
        s_cc = ctx.enter_context(nc.semaphore("s_cc"))
        s_cs = [ctx.enter_context(nc.semaphore(f"s_cs{h}")) for h in range(H)]
        s_stt = ctx.enter_context(nc.semaphore("s_stt"))
        s_souts = ctx.enter_context(nc.semaphore("s_souts"))
        block = ctx.enter_context(nc.Block())

        @block.gpsimd
        def _(g):
            for t in range(tt):
                g.dma_start(tiles[t][:, :], mx_l[t, :, :]).then_inc(s_in[t], 16)
            g.wait_ge(s_ccin, 16)
            g.collective_compute(
                "AllGather",
                mybir.AluOpType.bypass,
                replica_groups=[list(range(ncores))],
                ins=[cc_in[:]],
                outs=[cc_out[:]],
            ).then_inc(s_cc, 1)


        @block.sync
        def _(sp):
            sp.dma_start(eye_sb[:, :], eye[:, :]).then_inc(s_eye, 16)
            # local r_inv' (transposed) -> DRAM for the AllGather
            sp.wait_ge(s_ptc, 1)
            sp.dma_start(cc_in[:], ptc[:, :]).then_inc(s_ccin, 16)
            if debug:
                sp.wait_ge(s_sqd, 1)
                sp.dma_start(o_psa[:, :], psa[:, :]).then_inc(s_souts, 16)
                sp.dma_start(o_psbs[:, :], psbs[:, :]).then_inc(s_souts, 16)
                sp.dma_start(o_rsqh[:, :], rsqh[:, :]).then_inc(s_souts, 16)
                sp.wait_ge(s_ccin, 16)
                sp.dma_start(o_ccin[:], cc_in[:]).then_inc(s_souts, 16)
                sp.wait_ge(s_cs[H - 1], 16)
                sp.dma_start(o_ccout[:], cc_out[:]).then_inc(s_souts, 16)
                sp.dma_start(o_cs[:, :], colscale[:, :]).then_inc(s_souts, 16)
                sp.wait_ge(s_souts, 96)
            # stores: tile-half k as soon as its col-scale lands
            k = 0
            extra = 96 if debug else 0
            for h in range(H):
                for t in range(tt):
                    k += 1
                    sp.wait_ge(s_stt, k)
                    sp.dma_start(
                        out_v[t, :, h], tiles[t][:, h * w : (h + 1) * w]
                    ).then_inc(s_souts, 16)
            sp.wait_ge(s_souts, 16 * tt * H + extra)

        @block.scalar
        def _(s):
            # rowsum half A per tile: in-place Copy with f32 accum
            for t in range(tt):
                s.wait_ge(s_in[t], 16)
                s.activation(
                    tiles[t][:, 0:CA],
                    tiles[t][:, 0:CA],
                    mybir.ActivationFunctionType.Copy,
                    accum_out=psa[:, t : t + 1],
                ).then_inc(s_redA, 1)
            # drain own accum writebacks (self-wait), then publish: the
            # dummy op's sem increment cannot fire before the drain, so a
            # cross-engine reader of psa gated on s_redAd is safe
            s.wait_ge(s_redA, tt)
            s.activation(
                dr1[:, :], psa[:, tt - 1 : tt],
                mybir.ActivationFunctionType.Copy,
            ).then_inc(s_redAd, 1)
            # rsq' = sqrt(rowsum * 2^-26)  (fp16 value ~0.7)
            s.wait_ge(s_ps, 1)
            s.activation(
                rsqh[:, :],
                ps[:, :],
                mybir.ActivationFunctionType.Sqrt,
                scale=1.0 / SCALE_OUT,
            ).then_inc(s_sq, 1)
            # drain + publish rsqh the same way for PE/DVE readers
            s.wait_ge(s_sq, 1)
            s.activation(
                dr2[:, :], rsqh[:, tt - 1 : tt],
                mybir.ActivationFunctionType.Copy,
            ).then_inc(s_sqd, 1)
            # colscale broadcast, in halves so pass 2 starts on half 0
            # (this engine's HW-DGE ring completes small DMAs faster than
            # the gpsimd SWDGE ring, and it is idle by now)
            s.wait_ge(s_cc, 1)
            for h in range(H):
                s.dma_start(
                    colscale[:, h * w : (h + 1) * w],
                    cc_out[h * w : (h + 1) * w].partition_broadcast(P),
                ).then_inc(s_cs[h], 16)

        @block.tensor
        def _(pe):
            pe.wait_ge(s_eye, 16)
            pe.wait_ge(s_sqd, 1)
            pe.transpose(pt[:, :], rsqh[:, :], eye_sb[:, :]).then_inc(s_tpl, 1)

        @block.vector
        def _(v):
            # rowsum half B per tile
            for t in range(tt):
                v.wait_ge(s_in[t], 16)
                v.tensor_reduce(
                    psb[:, t : t + 1],
                    tiles[t][:, CA:n],
                    mybir.AxisListType.X,
                    mybir.AluOpType.add,
                )
            # combine rowsum halves (psa safe to read after s_redAd)
            v.wait_ge(s_redAd, 1)
            v.tensor_tensor(
                ps[:, :], psa[:, :], psb[:, :], mybir.AluOpType.add
            ).then_inc(s_ps, 1)
            # row scalars first (f32: tensor_scalar mult requires a float32
            # scalar operand); rsqh safe after the ACT drain-publish
            v.wait_ge(s_sqd, 1)
            v.reciprocal(rx8[:, :], rsqh[:, :])
            with nc.allow_low_precision(reason="fp16 r_inv, tol 2e-2"):
                # transposed reciprocal straight out of PSUM -> cc payload
                v.wait_ge(s_tpl, 1)
                v.reciprocal(ptc[:, :], pt[:, :]).then_inc(s_ptc, 1)
            # row scale, in place, while the AllGather is in flight
            for t in range(tt):
                for h in range(H):
                    v.tensor_scalar_mul(
                        tiles[t][:, h * w : (h + 1) * w],
                        tiles[t][:, h * w : (h + 1) * w],
                        rx8[:, t : t + 1],
                    )
            # pass 2: column scale, in place, half 0 first
            for h in range(H):
                v.wait_ge(s_cs[h], 16)
                for t in range(tt):
                    v.tensor_tensor(
                        tiles[t][:, h * w : (h + 1) * w],
                        tiles[t][:, h * w : (h + 1) * w],
                        colscale[:, h * w : (h + 1) * w],
                        mybir.AluOpType.mult,
                    ).then_inc(s_stt, 1)

    return nc


_NC_CACHE = {}


def _get_nc(n=N, ncores=NCORES):
    key = (n, ncores)
    if key not in _NC_CACHE:
        _NC_CACHE[key] = build_kernel(n, ncores)
    return _NC_CACHE[key]


def kernel(adj, **run_kwargs):
    adj = np.asarray(adj)
    assert adj.shape == (N, N) and adj.dtype == np.float32
    mxh = (adj * SCALE_IN).astype(np.float16)
    idx = np.arange(N)
    mxh[idx, idx] = (
        adj[idx, idx].astype(np.float64) * SCALE_IN + SCALE_IN
    ).astype(np.float16)
    eye = np.eye(P, dtype=np.float16)

    in_maps = [
        {"mx": mxh[c * SHARD : (c + 1) * SHARD], "eye": eye}
        for c in range(NCORES)
    ]
    nc = _get_nc()
    try:
        res = run_bass_kernel_spmd(nc, in_maps, list(range(NCORES)), **run_kwargs)
    except Exception:
        import time

        time.sleep(2.0)
        res = run_bass_kernel_spmd(nc, in_maps, list(range(NCORES)), **run_kwargs)

    full = np.concatenate(
        [res.results[c]["out"].astype(np.float32) for c in range(NCORES)],
        axis=0,
    ) / SCALE_OUT
    if run_kwargs:
        return full, res
    return full



# revision 3
# speedup vs baseline: 1.0640x; 1.0640x over previous
"""Normalized-adjacency kernel (EstimateAdj.normalize, symmetric=False) for TRN2.

out = mx * r_inv[:, None] * r_inv[None, :]   where mx = adj + I,
r_inv = rowsum(mx) ** -0.5.

Strategy (8 NeuronCores, raw Bass, fp16 data movement), v2 "chunked-AG":
  - host: mx' = (adj + I) * 2^13 cast to fp16; INTERLEAVED row sharding:
    core c owns global rows {t*1024 + c*128 + p : t in 0..7, p in 0..127},
    i.e. tile t of core c is the global 128-row band t*1024 + c*128.
    With this map, "tiles 0-3 of every core" = global rows [0, 4096) =
    a contiguous half of the COLUMNS for the later column scale.
  - device, per core (shard 1024 x 8192 fp16 = 16 MiB resident in SBUF):
      8 tile loads [128 x 8192] on the gpsimd SWDGE ring (~335 GB/s).
      rowsum per tile split DVE (cols [0:DV), tensor_reduce, 1x mode) /
      ACT (cols [DV:], Copy + f32 accum); DV enlarged for tiles 3 and 7
      so the chunk-closing reduce finishes ~1 us sooner.
      The AllGather is split in TWO chunks (tiles 0-3, tiles 4-7): each
      chunk's r_inv' (= 1/sqrt(rowsum * 2^-26), fp16) goes ACT sqrt ->
      PE transpose -> DVE reciprocal -> SWDGE DMA -> AllGather_k, so
      AG0 (trigger ~42 us) runs UNDER the load phase and only AG1
      (trigger ~67 us, ring ~22 us) remains on the critical path.
      Row scale: tiles 0-3 on DVE (tensor_scalar 4x) interleaved with
      the chunk-1 reduces; tiles 4-7 on ACT (Copy + per-partition scale)
      during the AG1 window, batch drain-published per column half.
      Column scale: colscale_k = partition_broadcast of the raw AG_k
      output (contiguous); the (c,t,p) -> (t,c,p) permutation between
      AG concat order and global column order is folded into the
      tensor_tensor src1 access pattern (inner 128-elem runs stay
      contiguous so the TT keeps 2x packed mode).
      bcast0 goes on the sync HWDGE ring (idle), bcast1 on the ACT ring.
      Stores: 16 x 1 MiB per-tile-half on the sync ring, each gated on
      its TT. ACT->consumer hops use the drain-publish idiom (self-wait
      then dummy-op publish) to avoid stale-writeback reads.
  - host: upcast, divide by 2^26, un-interleave rows.
  Baseline (single AG after all loads): ~164 us. This layout targets
  ~122-127 us: loads 9->60, AG0 hidden, AG1 67->89, stores 70->122.

(remote_dma peer-write exchange would cut the collective further but
InstRemoteDMA*/hostgen variants fail neuronxcc walrus codegen on this
toolchain: "ISA wrong length" in CoreV2GenImpl visitInstISA.)
"""

from contextlib import ExitStack

import numpy as np

import concourse.bass as bass
import concourse.mybir as mybir
from concourse.bass_utils import run_bass_kernel_spmd

N = 8192
NCORES = 8
SHARD = N // NCORES  # 1024
P = 128
T = SHARD // P  # 8 tiles per core
H = 2  # column halves (chunk granularity), 4096 cols each
CT = T // 2  # tiles per chunk (4)
W = N // H  # 4096
DV = 1472  # DVE rowsum columns for non-closing tiles (ACT takes the rest)
DVC = 2880  # balanced split for the chunk-closing tiles (3 and 7)

F16 = mybir.dt.float16
F32 = mybir.dt.float32

SCALE_IN = 8192.0  # 2^13
SCALE_OUT = float(2**26)


def build_kernel(n=N, ncores=NCORES):
    shard = n // ncores
    tt = shard // P  # 8
    w = n // H  # 4096
    ct = tt // 2  # 4

    nc = bass.Bass(num_devices=ncores)
    mx = nc.dram_tensor("mx", [shard, n], F16, kind="ExternalInput")
    eye = nc.dram_tensor("eye", [P, P], F16, kind="ExternalInput")
    out = nc.dram_tensor("out", [shard, n], F16, kind="ExternalOutput")
    cc_in = [nc.dram_tensor(f"cc_in{k}", [ct * P], F16) for k in range(H)]
    cc_out = [
        nc.dram_tensor(f"cc_out{k}", [ncores * ct * P], F16, addr_space="Shared")
        for k in range(H)
    ]

    mx_l = mx.rearrange("(t p) m -> t p m", p=P)
    out_v = out.rearrange("(t p) (h w) -> t p h w", p=P, h=H)

    with ExitStack() as ctx:
        tiles = [
            ctx.enter_context(nc.sbuf_tensor(f"tile{t}", [P, n], F16))
            for t in range(tt)
        ]
        colscale = ctx.enter_context(nc.sbuf_tensor("colscale", [P, n], F16))
        eye_sb = ctx.enter_context(nc.sbuf_tensor("eye_sb", [P, P], F16))
        psa = ctx.enter_context(nc.sbuf_tensor("psa", [P, tt], F32))
        psb = ctx.enter_context(nc.sbuf_tensor("psb", [P, tt], F32))
        ps = ctx.enter_context(nc.sbuf_tensor("ps", [P, tt], F32))
        dr1 = ctx.enter_context(nc.sbuf_tensor("dr1", [P, 1], F32))
        dr2 = ctx.enter_context(nc.sbuf_tensor("dr2", [P, 1], F16))
        rsqh = ctx.enter_context(nc.sbuf_tensor("rsqh", [P, tt], F16))
        rx8 = ctx.enter_context(nc.sbuf_tensor("rx8", [P, tt], F32))
        ptc = [
            ctx.enter_context(nc.sbuf_tensor(f"ptc{k}", [ct, P], F16))
            for k in range(H)
        ]
        pt = [
            ctx.enter_context(nc.psum_tensor(f"pt{k}", [ct, P], F16))
            for k in range(H)
        ]

        s_in = [ctx.enter_context(nc.semaphore(f"s_in{t}")) for t in range(tt)]
        s_eye = ctx.enter_context(nc.semaphore("s_eye"))
        s_redA = ctx.enter_context(nc.semaphore("s_redA"))  # raw ACT accum count
        s_redAd = [ctx.enter_context(nc.semaphore(f"s_redAd{k}")) for k in range(H)]
        s_ps = [ctx.enter_context(nc.semaphore(f"s_ps{k}")) for k in range(H)]
        s_sq = ctx.enter_context(nc.semaphore("s_sq"))  # raw sqrt count
        s_sqd = [ctx.enter_context(nc.semaphore(f"s_sqd{k}")) for k in range(H)]
        s_tpl = [ctx.enter_context(nc.semaphore(f"s_tpl{k}")) for k in range(H)]
        s_ptc = [ctx.enter_context(nc.semaphore(f"s_ptc{k}")) for k in range(H)]
        s_ccin = [ctx.enter_context(nc.semaphore(f"s_ccin{k}")) for k in range(H)]
        s_cc = [ctx.enter_context(nc.semaphore(f"s_cc{k}")) for k in range(H)]
        s_cs = [ctx.enter_context(nc.semaphore(f"s_cs{k}")) for k in range(H)]
        s_tsr = ctx.enter_context(nc.semaphore("s_tsr"))  # raw ACT row-scale count
        s_ts = [ctx.enter_context(nc.semaphore(f"s_ts{k}")) for k in range(H)]
        s_stt = ctx.enter_context(nc.semaphore("s_stt"))
        s_souts = ctx.enter_context(nc.semaphore("s_souts"))
        block = ctx.enter_context(nc.Block())

        def dvcols(t):
            return DVC if t % ct == ct - 1 else DV

        # permuted view of the raw-broadcast AG output for chunk k:
        # raw index (c, t2, p); global column order within the half is
        # (t2, c, p). Inner 128 runs stay contiguous.
        def colscale_perm(k):
            return colscale[:, k * w : (k + 1) * w].rearrange(
                "q (c t p) -> q t c p", c=ncores, t=ct, p=P
            )

        def tile_half_v(t, k):
            return tiles[t][:, k * w : (k + 1) * w].rearrange(
                "q (t c p) -> q t c p", t=ct, c=ncores, p=P
            )

        @block.gpsimd
        def _(g):
            for t in range(tt):
                g.dma_start(tiles[t][:, :], mx_l[t, :, :]).then_inc(s_in[t], 16)
            for k in range(H):
                # payload: r_inv' for local tiles [k*ct, (k+1)*ct), t-major
                g.wait_ge(s_ptc[k], 1)
                g.dma_start(cc_in[k][:], ptc[k][:, :]).then_inc(s_ccin[k], 16)
                g.wait_ge(s_ccin[k], 16)
                g.collective_compute(
                    "AllGather",
                    mybir.AluOpType.bypass,
                    replica_groups=[list(range(ncores))],
                    ins=[cc_in[k][:]],
                    outs=[cc_out[k][:]],
                ).then_inc(s_cc[k], 1)

        @block.sync
        def _(sp):
            sp.dma_start(eye_sb[:, :], eye[:, :]).then_inc(s_eye, 16)
            # colscale broadcast for chunk 0 (sync HWDGE ring is idle here)
            sp.wait_ge(s_cc[0], 1)
            sp.dma_start(
                colscale[:, 0:w], cc_out[0][:].partition_broadcast(P)
            ).then_inc(s_cs[0], 16)
            # stores: tile-half as soon as its col-scale TT lands
            k = 0
            for h in range(H):
                for t in range(tt):
                    k += 1
                    sp.wait_ge(s_stt, k)
                    sp.dma_start(
                        out_v[t, :, h], tiles[t][:, h * w : (h + 1) * w]
                    ).then_inc(s_souts, 16)
            sp.wait_ge(s_souts, 16 * tt * H)

        @block.scalar
        def _(s):
            for k in range(H):
                # rowsum high-cols per tile: in-place Copy with f32 accum
                for t in range(k * ct, (k + 1) * ct):
                    s.wait_ge(s_in[t], 16)
                    s.activation(
                        tiles[t][:, dvcols(t) : n],
                        tiles[t][:, dvcols(t) : n],
                        mybir.ActivationFunctionType.Copy,
                        accum_out=psa[:, t : t + 1],
                    ).then_inc(s_redA, 1)
                # drain own accum writebacks (self-wait), then publish
                s.wait_ge(s_redA, (k + 1) * ct)
                s.activation(
                    dr1[:, :],
                    psa[:, (k + 1) * ct - 1 : (k + 1) * ct],
                    mybir.ActivationFunctionType.Copy,
                ).then_inc(s_redAd[k], 1)
                # rsq' = sqrt(rowsum * 2^-26)  (fp16 value ~0.7)
                s.wait_ge(s_ps[k], 1)
                s.activation(
                    rsqh[:, k * ct : (k + 1) * ct],
                    ps[:, k * ct : (k + 1) * ct],
                    mybir.ActivationFunctionType.Sqrt,
                    scale=1.0 / SCALE_OUT,
                ).then_inc(s_sq, 1)
                # drain + publish rsqh for PE/DVE readers
                s.wait_ge(s_sq, k + 1)
                s.activation(
                    dr2[:, :],
                    rsqh[:, (k + 1) * ct - 1 : (k + 1) * ct],
                    mybir.ActivationFunctionType.Copy,
                ).then_inc(s_sqd[k], 1)
            # row scale for tiles 4..7 (chunk 1) on ACT while it is
            # otherwise idle in the AG1 window; h-half-major so the DVE
            # col-scale TTs of half 0 are never gated on half-1 work.
            # rx8 (DVE) is safe to read once s_ptc[1] has fired: DVE
            # computes rx8[ct:] before ptc[1] in its program order.
            s.wait_ge(s_ptc[1], 1)
            for h in range(H):
                for t in range(ct, tt):
                    s.activation(
                        tiles[t][:, h * w : (h + 1) * w],
                        tiles[t][:, h * w : (h + 1) * w],
                        mybir.ActivationFunctionType.Copy,
                        scale=rx8[:, t : t + 1],
                    ).then_inc(s_tsr, 1)
                # drain + publish the half's row scales
                s.wait_ge(s_tsr, (h + 1) * ct)
                s.activation(
                    dr2[:, :],
                    rsqh[:, tt - 1 : tt],
                    mybir.ActivationFunctionType.Copy,
                ).then_inc(s_ts[h], 1)
            # colscale broadcast for chunk 1 (ACT HWDGE ring; free now)
            s.wait_ge(s_cc[1], 1)
            s.dma_start(
                colscale[:, w : 2 * w], cc_out[1][:].partition_broadcast(P)
            ).then_inc(s_cs[1], 16)

        @block.tensor
        def _(pe):
            pe.wait_ge(s_eye, 16)
            for k in range(H):
                pe.wait_ge(s_sqd[k], 1)
                pe.transpose(
                    pt[k][:, :], rsqh[:, k * ct : (k + 1) * ct], eye_sb[:, :]
                ).then_inc(s_tpl[k], 1)

        @block.vector
        def _(v):
            def red(t):
                v.wait_ge(s_in[t], 16)
                v.tensor_reduce(
                    psb[:, t : t + 1],
                    tiles[t][:, 0 : dvcols(t)],
                    mybir.AxisListType.X,
                    mybir.AluOpType.add,
                )

            def chunk_chain(k):
                # combine rowsum halves (psa safe after ACT drain-publish)
                v.wait_ge(s_redAd[k], 1)
                v.tensor_tensor(
                    ps[:, k * ct : (k + 1) * ct],
                    psa[:, k * ct : (k + 1) * ct],
                    psb[:, k * ct : (k + 1) * ct],
                    mybir.AluOpType.add,
                ).then_inc(s_ps[k], 1)
                # row-scale scalars for this chunk (f32 for tensor_scalar;
                # also read by ACT for tiles 4-7 once s_ptc[1] fires)
                v.wait_ge(s_sqd[k], 1)
                v.reciprocal(
                    rx8[:, k * ct : (k + 1) * ct], rsqh[:, k * ct : (k + 1) * ct]
                )
                with nc.allow_low_precision(reason="fp16 r_inv, tol 2e-2"):
                    # transposed reciprocal straight out of PSUM -> AG payload
                    v.wait_ge(s_tpl[k], 1)
                    v.reciprocal(ptc[k][:, :], pt[k][:, :]).then_inc(s_ptc[k], 1)

            def rowscale(t):
                for hh in range(H):
                    v.tensor_scalar_mul(
                        tiles[t][:, hh * w : (hh + 1) * w],
                        tiles[t][:, hh * w : (hh + 1) * w],
                        rx8[:, t : t + 1],
                    )

            # chunk 0: reduces + r_inv chain -> AG0 trigger ~42 us
            for t in range(ct):
                red(t)
            chunk_chain(0)
            # chunk 1 reduces, interleaved with chunk-0 row scales so the
            # chunk-1 chain is never delayed behind bulk TS work
            red(ct)
            rowscale(0)
            red(ct + 1)
            rowscale(1)
            red(ct + 2)
            rowscale(2)
            red(ct + 3)
            chunk_chain(1)
            rowscale(3)
            # column scale, half 0 then half 1; tiles 4-7's row scale runs
            # on ACT, gated per half via s_ts
            for h in range(H):
                v.wait_ge(s_cs[h], 16)
                for t in range(tt):
                    if t == ct:
                        # tiles 4-7 were row-scaled on ACT; wait for the
                        # half's batch drain-publish before reading them
                        v.wait_ge(s_ts[h], 1)
                    v.tensor_tensor(
                        tile_half_v(t, h),
                        tile_half_v(t, h),
                        colscale_perm(h),
                        mybir.AluOpType.mult,
                    ).then_inc(s_stt, 1)

    return nc


_NC_CACHE = {}


def _get_nc(n=N, ncores=NCORES):
    key = (n, ncores)
    if key not in _NC_CACHE:
        _NC_CACHE[key] = build_kernel(n, ncores)
    return _NC_CACHE[key]


def kernel(adj, **run_kwargs):
    adj = np.asarray(adj)
    assert adj.shape == (N, N) and adj.dtype == np.float32
    mxh = (adj * SCALE_IN).astype(np.float16)
    idx = np.arange(N)
    mxh[idx, idx] = (
        adj[idx, idx].astype(np.float64) * SCALE_IN + SCALE_IN
    ).astype(np.float16)
    eye = np.eye(P, dtype=np.float16)

    # interleaved sharding: core c's tile t = global rows t*1024 + c*128
    mxv = mxh.reshape(T, NCORES, P, N)
    in_maps = [
        {"mx": np.ascontiguousarray(mxv[:, c]).reshape(SHARD, N), "eye": eye}
        for c in range(NCORES)
    ]
    nc = _get_nc()
    try:
        res = run_bass_kernel_spmd(nc, in_maps, list(range(NCORES)), **run_kwargs)
    except Exception:
        import time

        time.sleep(2.0)
        res = run_bass_kernel_spmd(nc, in_maps, list(range(NCORES)), **run_kwargs)

    full = np.empty((T, NCORES, P, N), dtype=np.float32)
    for c in range(NCORES):
        full[:, c] = (
            res.results[c]["out"].astype(np.float32).reshape(T, P, N)
        )
    full = full.reshape(N, N) / SCALE_OUT
    if run_kwargs:
        return full, res
    return full


# revision 14
# speedup vs baseline: 1.1066x; 1.0401x over previous
"""Normalized-adjacency kernel (EstimateAdj.normalize, symmetric=False) for TRN2.

out = mx * r_inv[:, None] * r_inv[None, :]   where mx = adj + I,
r_inv = rowsum(mx) ** -0.5.

Strategy (8 NeuronCores, raw Bass, fp16 data movement), v4 "chunked-AG":
  - host: mx' = (adj + I) * 2^13 cast to fp16; INTERLEAVED row sharding:
    core c owns global rows {t*1024 + c*128 + p}, i.e. tile t of core c is
    the global 128-row band t*1024 + c*128. With this map, "tiles 0-3 of
    every core" = global rows [0, 4096) = a contiguous half of the COLUMNS
    for the later column scale, so the AllGather can be split in two chunks
    that pipeline with the load phase without fragmenting the stores.
  - device, per core (shard 1024 x 8192 fp16 = 16 MiB resident in SBUF):
      8 tile loads [128 x 8192] on the gpsimd SWDGE ring (~335 GB/s).
      rowsum per tile split ACT prefix [0:AC) (Copy + f32 accum - MUST
      start at column 0: a non-zero source offset drops the ACT Copy from
      2x to 1x mode, measured 7.1 us vs 3.3 us) / DVE suffix [AC:) via
      tensor_reduce; AC shrunk for the chunk-closing tiles 3 and 7.
      Per-chunk r_inv' chain: DVE comb (psa+psb) -> ACT sqrt(ps * 2^-26)
      (drain-publish) -> PE transpose [128,4]->[4,128] -> DVE reciprocal
      out of PSUM -> 1 KiB payload DMA on the SYNC ring (NOT the gpsimd
      ring, where it would FIFO behind the remaining 2 MiB tile loads -
      measured +14 us) -> AllGather chunk k (1 KiB/rank).
      AG0 (doorbell ~47 us) runs under the load phase; AG1 (doorbell
      ~73 us) pays ~13 us ncfw entry + ~13-18 us ring, serialized on the
      CC cores after AG0.
      Row scale: tiles 0-3 on DVE (tensor_scalar 4x) interleaved with the
      chunk-1 reduces; tiles 4-7 on ACT (Copy + per-partition scale) in
      the AG1 window, h-half-major, batch drain-published per half (s_ts).
      Column scale: colscale_k = partition_broadcast of the raw AG_k
      output (contiguous 1 MiB DRE write; bcast0 on the sync ring, bcast1
      on the ACT ring); the (c,t,p)->(t,c,p) permutation between AG concat
      order and global column order is folded into the tensor_tensor src1
      access pattern (inner 128-elem runs stay contiguous, TT keeps 2x -
      verified 2.28 us per half-tile in the v2 trace).
      Stores: 16 x 1 MiB per-tile-half on the sync ring, gated per TT;
      h0 stores (s_cc0 + ~7 us) drain while AG1 is still in flight.
  - host: upcast, divide by 2^26, un-interleave rows.
  Measured: baseline single-AG 164 us; v2 154 us (payload DMAs on the
  gpsimd ring ate 14 us; ACT at 1x ate the load-phase slack).

(remote_dma peer-write exchange would cut the collective further but
InstRemoteDMA*/hostgen variants fail neuronxcc walrus codegen on this
toolchain: "ISA wrong length" in CoreV2GenImpl visitInstISA.)
"""

from contextlib import ExitStack

import numpy as np

import concourse.bass as bass
import concourse.mybir as mybir
from concourse.bass_utils import run_bass_kernel_spmd

N = 8192
NCORES = 8
SHARD = N // NCORES  # 1024
P = 128
T = SHARD // P  # 8 tiles per core
H = 2  # column halves (chunk granularity), 4096 cols each
CT = T // 2  # tiles per chunk (4)
W = N // H  # 4096
AC = 5440  # ACT rowsum prefix columns (DVE takes the suffix)
ACC = 4864  # smaller ACT prefix for the chunk-closing tiles (3 and 7)

F16 = mybir.dt.float16
F32 = mybir.dt.float32

SCALE_IN = 8192.0  # 2^13
SCALE_OUT = float(2**26)


def build_kernel(n=N, ncores=NCORES):
    shard = n // ncores
    tt = shard // P  # 8
    w = n // H  # 4096
    ct = tt // 2  # 4

    nc = bass.Bass(num_devices=ncores)
    mx = nc.dram_tensor("mx", [shard, n], F16, kind="ExternalInput")
    eye = nc.dram_tensor("eye", [P, P], F16, kind="ExternalInput")
    out = nc.dram_tensor("out", [shard, n], F16, kind="ExternalOutput")
    cc_in = [nc.dram_tensor(f"cc_in{k}", [ct * P], F16) for k in range(H)]
    cc_out = [
        nc.dram_tensor(f"cc_out{k}", [ncores * ct * P], F16, addr_space="Shared")
        for k in range(H)
    ]

    mx_l = mx.rearrange("(t p) m -> t p m", p=P)
    out_v = out.rearrange("(t p) (h w) -> t p h w", p=P, h=H)

    with ExitStack() as ctx:
        tiles = [
            ctx.enter_context(nc.sbuf_tensor(f"tile{t}", [P, n], F16))
            for t in range(tt)
        ]
        colscale = ctx.enter_context(nc.sbuf_tensor("colscale", [P, n], F16))
        eye_sb = ctx.enter_context(nc.sbuf_tensor("eye_sb", [P, P], F16))
        psa = ctx.enter_context(nc.sbuf_tensor("psa", [P, tt], F32))
        psb = ctx.enter_context(nc.sbuf_tensor("psb", [P, tt], F32))
        ps = ctx.enter_context(nc.sbuf_tensor("ps", [P, tt], F32))
        dr1 = ctx.enter_context(nc.sbuf_tensor("dr1", [P, 1], F32))
        dr3 = ctx.enter_context(nc.sbuf_tensor("dr3", [P, 1], F32))
        dr2 = ctx.enter_context(nc.sbuf_tensor("dr2", [P, 1], F16))
        rsqh = ctx.enter_context(nc.sbuf_tensor("rsqh", [P, tt], F16))
        rx8 = ctx.enter_context(nc.sbuf_tensor("rx8", [P, tt], F32))
        ptc = [
            ctx.enter_context(nc.sbuf_tensor(f"ptc{k}", [ct, P], F16))
            for k in range(H)
        ]
        pt = [
            ctx.enter_context(nc.psum_tensor(f"pt{k}", [ct, P], F16))
            for k in range(H)
        ]

        s_in = [ctx.enter_context(nc.semaphore(f"s_in{t}")) for t in range(tt)]
        s_eye = ctx.enter_context(nc.semaphore("s_eye"))
        s_rdv = ctx.enter_context(nc.semaphore("s_rdv"))  # DVE reduce count
        s_redA = ctx.enter_context(nc.semaphore("s_redA"))  # raw ACT accum count
        s_redAd = [ctx.enter_context(nc.semaphore(f"s_redAd{k}")) for k in range(H)]
        s_ps = [ctx.enter_context(nc.semaphore(f"s_ps{k}")) for k in range(H)]
        s_sq = ctx.enter_context(nc.semaphore("s_sq"))  # raw sqrt count
        s_sqd = [ctx.enter_context(nc.semaphore(f"s_sqd{k}")) for k in range(H)]
        s_tpl = [ctx.enter_context(nc.semaphore(f"s_tpl{k}")) for k in range(H)]
        s_ptc = [ctx.enter_context(nc.semaphore(f"s_ptc{k}")) for k in range(H)]
        s_ccin = [ctx.enter_context(nc.semaphore(f"s_ccin{k}")) for k in range(H)]
        s_cc = [ctx.enter_context(nc.semaphore(f"s_cc{k}")) for k in range(H)]
        s_cs = [ctx.enter_context(nc.semaphore(f"s_cs{k}")) for k in range(H)]
        s_tsr = ctx.enter_context(nc.semaphore("s_tsr"))  # raw ACT row-scale
        s_ts = [ctx.enter_context(nc.semaphore(f"s_ts{k}")) for k in range(H)]
        s_stt = ctx.enter_context(nc.semaphore("s_stt"))
        s_souts = ctx.enter_context(nc.semaphore("s_souts"))
        block = ctx.enter_context(nc.Block())

        def accols(t):
            return ACC if t % ct == ct - 1 else AC

        # permuted view of the raw-broadcast AG output for chunk k:
        # raw index (c, t2, p); global column order within the half is
        # (t2, c, p). Inner 128 runs stay contiguous.
        def colscale_perm(k):
            return colscale[:, k * w : (k + 1) * w].rearrange(
                "q (c t p) -> q t c p", c=ncores, t=ct, p=P
            )

        def tile_half_v(t, k):
            return tiles[t][:, k * w : (k + 1) * w].rearrange(
                "q (t c p) -> q t c p", t=ct, c=ncores, p=P
            )

        @block.gpsimd
        def _(g):
            for t in range(tt):
                g.dma_start(tiles[t][:, :], mx_l[t, :, :]).then_inc(s_in[t], 16)
            for k in range(H):
                g.wait_ge(s_ccin[k], 16)
                g.collective_compute(
                    "AllGather",
                    mybir.AluOpType.bypass,
                    replica_groups=[list(range(ncores))],
                    ins=[cc_in[k][:]],
                    outs=[cc_out[k][:]],
                ).then_inc(s_cc[k], 1)

        @block.sync
        def _(sp):
            sp.dma_start(eye_sb[:, :], eye[:, :]).then_inc(s_eye, 16)
            # AG payloads (1 KiB each) on the sync HWDGE ring, ahead of the
            # stores in program order
            for k in range(H):
                sp.wait_ge(s_ptc[k], 1)
                sp.dma_start(cc_in[k][:], ptc[k][:, :]).then_inc(s_ccin[k], 16)
            # colscale broadcast for chunk 0 (sync ring is idle here)
            sp.wait_ge(s_cc[0], 1)
            sp.dma_start(
                colscale[:, 0:w], cc_out[0][:].partition_broadcast(P)
            ).then_inc(s_cs[0], 16)
            # half-0 stores on the sync ring (half-1 goes on the ACT ring so
            # the two HWDGE rings drain concurrently)
            for t in range(tt):
                sp.wait_ge(s_stt, t + 1)
                sp.dma_start(out_v[t, :, 0], tiles[t][:, 0:w]).then_inc(
                    s_souts, 16
                )
            sp.wait_ge(s_souts, 16 * tt * H)

        @block.scalar
        def _(s):
            for k in range(H):
                # rowsum prefix per tile: in-place Copy with f32 accum
                # (source offset 0 keeps the 2x perf mode)
                for t in range(k * ct, (k + 1) * ct):
                    s.wait_ge(s_in[t], 16)
                    s.activation(
                        tiles[t][:, 0 : accols(t)],
                        tiles[t][:, 0 : accols(t)],
                        mybir.ActivationFunctionType.Copy,
                        accum_out=psa[:, t : t + 1],
                    ).then_inc(s_redA, 1)
                # drain own accum writebacks (self-wait), then publish
                s.wait_ge(s_redA, (k + 1) * ct)
                s.activation(
                    dr1[:, :],
                    psa[:, (k + 1) * ct - 1 : (k + 1) * ct],
                    mybir.ActivationFunctionType.Copy,
                ).then_inc(s_redAd[k], 1)
                # rsq' = sqrt(rowsum * 2^-26)  (fp16 value ~0.7)
                s.wait_ge(s_ps[k], 1)
                s.activation(
                    rsqh[:, k * ct : (k + 1) * ct],
                    ps[:, k * ct : (k + 1) * ct],
                    mybir.ActivationFunctionType.Sqrt,
                    scale=1.0 / SCALE_OUT,
                ).then_inc(s_sq, 1)
                # drain + publish rsqh for PE/DVE readers
                s.wait_ge(s_sq, k + 1)
                s.activation(
                    dr2[:, :],
                    rsqh[:, (k + 1) * ct - 1 : (k + 1) * ct],
                    mybir.ActivationFunctionType.Copy,
                ).then_inc(s_sqd[k], 1)
            # row scale for tiles 4..7 on ACT while it is otherwise idle in
            # the AG1 window; h-half-major so DVE's half-0 TTs are gated
            # only on the first four. s_ptc[1] implies rx8[:, 4:8] is ready
            # (DVE computes rx8 chunk 1 before the ptc1 reciprocal).
            s.wait_ge(s_ptc[1], 1)
            for h in range(H):
                for t in range(ct, tt):
                    s.activation(
                        tiles[t][:, h * w : (h + 1) * w],
                        tiles[t][:, h * w : (h + 1) * w],
                        mybir.ActivationFunctionType.Copy,
                        scale=rx8[:, t : t + 1],
                    ).then_inc(s_tsr, 1)
                # drain + publish the half's row scales
                s.wait_ge(s_tsr, (h + 1) * ct)
                s.activation(
                    dr2[:, :],
                    rsqh[:, tt - 1 : tt],
                    mybir.ActivationFunctionType.Copy,
                ).then_inc(s_ts[h], 1)
            # colscale broadcast for chunk 1 (ACT HWDGE ring; free now)
            s.wait_ge(s_cc[1], 1)
            s.dma_start(
                colscale[:, w : 2 * w], cc_out[1][:].partition_broadcast(P)
            ).then_inc(s_cs[1], 16)
            # half-1 stores on the ACT ring, concurrent with sync's half-0
            for t in range(tt):
                s.wait_ge(s_stt, tt + t + 1)
                s.dma_start(out_v[t, :, 1], tiles[t][:, w:n]).then_inc(
                    s_souts, 16
                )

        @block.tensor
        def _(pe):
            pe.wait_ge(s_eye, 16)
            for k in range(H):
                pe.wait_ge(s_sqd[k], 1)
                pe.transpose(
                    pt[k][:, :], rsqh[:, k * ct : (k + 1) * ct], eye_sb[:, :]
                ).then_inc(s_tpl[k], 1)

        @block.vector
        def _(v):
            def red(t):
                v.wait_ge(s_in[t], 16)
                v.tensor_reduce(
                    psb[:, t : t + 1],
                    tiles[t][:, accols(t) : n],
                    mybir.AxisListType.X,
                    mybir.AluOpType.add,
                ).then_inc(s_rdv, 1)

            def chunk_chain(k):
                # tensor_reduce writebacks are lazy (accumulation path, like
                # ACT accum_out): self-wait until the closing reduce retires,
                # then a dummy read, before combining - otherwise the comb
                # can read a stale psb (measured: tiles 7 on 2 of 8 cores
                # lost the whole DVE partial when ACT was not the laggard)
                v.wait_ge(s_rdv, (k + 1) * ct)
                v.tensor_scalar_add(
                    dr3[:, :], psb[:, (k + 1) * ct - 1 : (k + 1) * ct], 0.0
                )
                # combine rowsum halves (psa safe after ACT drain-publish)
                v.wait_ge(s_redAd[k], 1)
                v.tensor_tensor(
                    ps[:, k * ct : (k + 1) * ct],
                    psa[:, k * ct : (k + 1) * ct],
                    psb[:, k * ct : (k + 1) * ct],
                    mybir.AluOpType.add,
                ).then_inc(s_ps[k], 1)
                # row-scale scalars (f32) while the PE transpose runs; must
                # precede the ptc reciprocal: ACT's tiles-4..7 row scale is
                # gated on s_ptc[1] and reads rx8[:, 4:8]
                v.wait_ge(s_sqd[k], 1)
                v.reciprocal(
                    rx8[:, k * ct : (k + 1) * ct], rsqh[:, k * ct : (k + 1) * ct]
                )
                with nc.allow_low_precision(reason="fp16 r_inv, tol 2e-2"):
                    # transposed reciprocal straight out of PSUM -> AG payload
                    v.wait_ge(s_tpl[k], 1)
                    v.reciprocal(ptc[k][:, :], pt[k][:, :]).then_inc(s_ptc[k], 1)

            def rowscale(t):
                for hh in range(H):
                    v.tensor_scalar_mul(
                        tiles[t][:, hh * w : (hh + 1) * w],
                        tiles[t][:, hh * w : (hh + 1) * w],
                        rx8[:, t : t + 1],
                    )

            # chunk 0: reduces + r_inv chain -> AG0 doorbell ~47 us
            for t in range(ct):
                red(t)
            chunk_chain(0)
            # chunk 1 reduces, interleaved with chunk-0 row scales; the
            # closing reduce stays unobstructed so AG1 fires on time
            red(ct)
            rowscale(0)
            red(ct + 1)
            rowscale(1)
            red(ct + 2)
            rowscale(2)
            red(ct + 3)
            chunk_chain(1)
            rowscale(3)
            # column scale, half 0 then half 1; tiles 4-7's row scale runs
            # on ACT, gated per half via s_ts
            for h in range(H):
                v.wait_ge(s_cs[h], 16)
                for t in range(tt):
                    if t == ct:
                        v.wait_ge(s_ts[h], 1)
                    v.tensor_tensor(
                        tile_half_v(t, h),
                        tile_half_v(t, h),
                        colscale_perm(h),
                        mybir.AluOpType.mult,
                    ).then_inc(s_stt, 1)

    return nc


_NC_CACHE = {}


def _get_nc(n=N, ncores=NCORES):
    key = (n, ncores)
    if key not in _NC_CACHE:
        _NC_CACHE[key] = build_kernel(n, ncores)
    return _NC_CACHE[key]


def kernel(adj, **run_kwargs):
    adj = np.asarray(adj)
    assert adj.shape == (N, N) and adj.dtype == np.float32
    mxh = (adj * SCALE_IN).astype(np.float16)
    idx = np.arange(N)
    mxh[idx, idx] = (
        adj[idx, idx].astype(np.float64) * SCALE_IN + SCALE_IN
    ).astype(np.float16)
    eye = np.eye(P, dtype=np.float16)

    # interleaved sharding: core c's tile t = global rows t*1024 + c*128
    mxv = mxh.reshape(T, NCORES, P, N)
    in_maps = [
        {"mx": np.ascontiguousarray(mxv[:, c]).reshape(SHARD, N), "eye": eye}
        for c in range(NCORES)
    ]
    nc = _get_nc()
    try:
        res = run_bass_kernel_spmd(nc, in_maps, list(range(NCORES)), **run_kwargs)
    except Exception:
        import time

        time.sleep(2.0)
        res = run_bass_kernel_spmd(nc, in_maps, list(range(NCORES)), **run_kwargs)

    full = np.empty((T, NCORES, P, N), dtype=np.float32)
    for c in range(NCORES):
        full[:, c] = (
            res.results[c]["out"].astype(np.float32).reshape(T, P, N)
        )
    full = full.reshape(N, N) / SCALE_OUT
    if run_kwargs:
        return full, res
    return full
